# revision 18
# baseline (speedup 1.0000x reference)
"""AttentionBlock kernel for Trainium2, data-parallel over batch across 8 NeuronCores.

Reference computation (per batch element b, with x viewed as [C, N] where N = H*W):
    xf   = x^T                                 # [N, C] tokens
    qkv  = xf @ W_qkv + b_qkv                  # [N, 3*h*d], h=8 heads, d=64
    S_h  = (q_h @ k_h^T) * d^-0.5              # [N, N] per head
    A_h  = softmax(S_h, axis=keys)
    res  = concat_h(A_h @ v_h) @ W_out + b_out + xf
    out  = res^T                               # [C, N]

Everything on-device is computed in the transposed (feature-major) layout so no
transposes are ever needed: x arrives as [C, N], qk^T = W_qk^T @ x, scores are
built directly as S^T = k^T q (keys on partitions), the softmax denominator
falls out of an appended ones-column in the v matmul, and the output projection
produces y^T = [C, N] which is exactly the DRAM output layout.

Matmuls run in float32r (fp32 fast mode, ~3x PE throughput vs fp32 on TRN2).
Engine split: PE matmuls; ACT does only the exp; DVE does PSUM->SBUF epilogues,
reciprocal and normalization; GPSIMD does fp32->fp32r staging casts and the
softmax-denominator partition broadcast.
"""

import numpy as np

import concourse.bass as bass
import concourse.tile as tile
from concourse import bacc, mybir
from concourse.bass_interp import get_hw_module
from concourse.bass_utils import run_bass_kernel_spmd

F32 = mybir.dt.float32
F32R = mybir.dt.float32r
EXP = mybir.ActivationFunctionType.Exp
IDENT = mybir.ActivationFunctionType.Identity

N_CORES = 8
B, C, H, W = 8, 512, 32, 32
N = H * W          # 1024 tokens
NH, D = 8, 64      # heads, head dim
SCALE = D ** -0.5  # 0.125

_cache = {}


def build_program(use_biases: bool):
    nc = bacc.Bacc("TRN2", target_bir_lowering=False, debug=False, num_devices=N_CORES)

    x_d = nc.dram_tensor("x", [C, N], F32, kind="ExternalInput").ap()
    wqk_d = nc.dram_tensor("wqk", [C, 2 * NH * D], F32, kind="ExternalInput").ap()
    wv_d = nc.dram_tensor("wv", [C, NH * D], F32, kind="ExternalInput").ap()
    wout_d = nc.dram_tensor("wout", [NH * D, C], F32, kind="ExternalInput").ap()
    bqk_d = nc.dram_tensor("bqk", [128, 8], F32, kind="ExternalInput").ap()
    bv_d = nc.dram_tensor("bv", [128, 4], F32, kind="ExternalInput").ap()
    bout_d = nc.dram_tensor("bout", [128, 4], F32, kind="ExternalInput").ap()
    y_d = nc.dram_tensor("y", [C, N], F32, kind="ExternalOutput").ap()

    with tile.TileContext(nc) as tc:
        _emit(nc, tc, x_d, wqk_d, wv_d, wout_d, bqk_d, bv_d, bout_d, y_d, use_biases)
    nc.compile()
    nc.m = get_hw_module(nc.m)
    return nc


def _emit(nc, tc, x_d, wqk_d, wv_d, wout_d, bqk_d, bv_d, bout_d, y_d, use_biases):
    import contextlib

    ctx = contextlib.ExitStack()
    with ctx:
        persist = ctx.enter_context(tc.tile_pool(name="persist", bufs=1))

        # ---- persistent SBUF tiles -------------------------------------
        x_r = [persist.tile([128, N], F32R, name=f"xr{i}", tag=f"xr{i}") for i in range(4)]
        wqk_r = [persist.tile([128, 1024], F32R, name=f"wqk{i}", tag=f"wqk{i}") for i in range(4)]
        wv_r = [persist.tile([128, 512], F32R, name=f"wv{i}", tag=f"wv{i}") for i in range(4)]
        wout_r = [persist.tile([128, 512], F32R, name=f"wout{i}", tag=f"wout{i}") for i in range(4)]
        # v with an appended ones column per head: [n-chunk][128, head, 65]
        v_sb = [persist.tile([128, NH, D + 1], F32R, name=f"v{i}", tag=f"v{i}") for i in range(8)]
        resT = [persist.tile([128, N], F32R, name=f"resT{i}", tag=f"resT{i}") for i in range(4)]
        ones_f = persist.tile([128, 64], F32, tag="ones_f")
        nc.vector.memset(ones_f[:], 1.0)

        if use_biases:
            bqk_sb = persist.tile([128, 8], F32, tag="bqk")
            bv_sb = persist.tile([128, 4], F32, tag="bv")
            bout_sb = persist.tile([128, 4], F32, tag="bout")
            nc.sync.dma_start(out=bqk_sb[:], in_=bqk_d[:])
            nc.sync.dma_start(out=bv_sb[:], in_=bv_d[:])
            nc.sync.dma_start(out=bout_sb[:], in_=bout_d[:])

        for i in range(8):
            nc.vector.tensor_copy(
                v_sb[i][:, :, D : D + 1],
                ones_f[:, 0:NH].rearrange("p (h o) -> p h o", o=1),
            )

        # ---- fused pipeline: loads, projections, attention ------------------
        with (
            tc.tile_pool(name="stage", bufs=2) as stage,
            tc.tile_pool(name="qkT_pool", bufs=4) as qkT_pool,
            tc.tile_pool(name="vtmp_pool", bufs=2) as vtmp_pool,
            tc.tile_pool(name="exp_pool", bufs=16) as exp_pool,
            tc.tile_pool(name="z_pool", bufs=3) as z_pool,
            tc.tile_pool(name="big_psum", bufs=2, space="PSUM") as big_psum,
            tc.tile_pool(name="v_psum", bufs=1, space="PSUM") as v_psum,
            tc.tile_pool(name="rest_psum", bufs=3, space="PSUM") as rest_psum,
        ):
            def load_round(dst, dram_ap, width, eng):
                t = stage.tile([128, 1024], F32, name="stage", tag="stage")
                eng.dma_start(out=t[:, 0:width], in_=dram_ap)
                nc.vector.tensor_copy(dst, t[:, 0:width])

            for i in range(4):
                load_round(x_r[i][:], x_d[bass.ts(i, 128), :], 1024, nc.sync)
            for i in range(4):
                load_round(wqk_r[i][:], wqk_d[bass.ts(i, 128), :], 1024, nc.scalar)
            for i in range(4):
                load_round(wv_r[i][:], wv_d[bass.ts(i, 128), :], 512, nc.gpsimd)
            for i in range(4):
                load_round(wout_r[i][:], wout_d[bass.ts(i, 128), :], 512, nc.gpsimd)

            def emit_qkT(m):
                # qkT[g, n] = sum_c wqk[c, g] x[c, n]
                qt = qkT_pool.tile([128, N], F32R, name="qkT", tag="qkT")
                qp = big_psum.tile([128, N], F32, name="qk", tag="big")
                for half in range(2):
                    for kc in range(4):
                        nc.tensor.matmul(
                            qp[:, bass.ts(half, 512)],
                            wqk_r[kc][:, bass.ts(m, 128)],
                            x_r[kc][:, bass.ts(half, 512)],
                            start=(kc == 0),
                            stop=(kc == 3),
                        )
                if use_biases:
                    nc.scalar.activation(
                        qt[:], qp[:], IDENT, bias=bqk_sb[:, m : m + 1]
                    )
                else:
                    nc.vector.tensor_copy(qt[:], qp[:])
                return qt

            def emit_v():
                # v_all[n, h*64+d] = sum_c x[c, n] wv[c, h*64+d]
                for nc_i in range(8):
                    vp = v_psum.tile([128, 512], F32, name="vp", tag="vp")
                    for kc in range(4):
                        nc.tensor.matmul(
                            vp[:],
                            x_r[kc][:, bass.ts(nc_i, 128)],
                            wv_r[kc][:],
                            start=(kc == 0),
                            stop=(kc == 3),
                        )
                    vt = vtmp_pool.tile([128, 512], F32R, name="vt", tag="vt")
                    nc.scalar.activation(vt[:], vp[:], IDENT)
                    nc.gpsimd.tensor_copy(
                        v_sb[nc_i][:, :, 0:D],
                        vt[:].rearrange("p (h d) -> p h d", h=NH),
                    )

            qkT_next = (emit_qkT(0), emit_qkT(4))
            for a in range(4):  # head pair (2a, 2a+1)
                qT_t, kT_t = qkT_next
                exps = {0: [], 64: []}
                # S^T = k^T q: head 2a on array rows 0-63, head 2a+1 on 64-127;
                # adjacent emission lets the PE run the pair concurrently.
                for jc in range(8):
                    sts = {}
                    for base in (0, 64):
                        sts[base] = big_psum.tile([128, N], F32, name="st", tag="big")
                    for ih in range(2):
                        for base in (0, 64):
                            nc.tensor.matmul(
                                sts[base][:, bass.ts(ih, 512)],
                                kT_t[base : base + 64, bass.ts(jc, 128)],
                                qT_t[base : base + 64, bass.ts(ih, 512)],
                                start=True,
                                stop=True,
                            )
                    for base in (0, 64):
                        e = exp_pool.tile([128, N], F32R, name="exp", tag="exp")
                        nc.scalar.activation(e[:], sts[base][:], EXP, scale=SCALE)
                        exps[base].append(e)
                if a < 3:
                    qkT_next = (emit_qkT(a + 1), emit_qkT(4 + a + 1))
                if a == 0:
                    emit_v()
                # res^T = v_aug . expS^T per head; psum row 64 = softmax denom Z
                for base in (0, 64):
                    h = 2 * a + (base // 64)
                    for ih in range(2):
                        rp = rest_psum.tile([D + 1, 512], F32, name="rest", tag="rest")
                        for jc in range(8):
                            nc.tensor.matmul(
                                rp[:],
                                v_sb[jc][:, h, :],
                                exps[base][jc][:, bass.ts(ih, 512)],
                                start=(jc == 0),
                                stop=(jc == 7),
                            )
                        # 1/Z: spread Z across 128 partitions so the DVE
                        # reciprocal runs on all lanes, then broadcast back.
                        zsb = z_pool.tile([1, 512], F32, name="zsb", tag="zsb")
                        nc.vector.tensor_copy(zsb[:], rp[D : D + 1, :])
                        zt = z_pool.tile([128, 4], F32, name="zt", tag="zt")
                        nc.sync.dma_start(
                            out=zt[:],
                            in_=zsb[:].rearrange("o (p f) -> o p f", p=128),
                        )
                        ztr = z_pool.tile([128, 4], F32, name="ztr", tag="ztr")
                        nc.vector.reciprocal(ztr[:], zt[:])
                        nc.sync.dma_start(
                            out=zsb[:].rearrange("o (p f) -> o p f", p=128),
                            in_=ztr[:],
                        )
                        zr = z_pool.tile([64, 512], F32, name="zr", tag="zr")
                        nc.gpsimd.partition_broadcast(zr[:], zsb[:])
                        out_slice = resT[a][base : base + 64, bass.ts(ih, 512)]
                        nc.vector.tensor_mul(out_slice, rp[0:D, :], zr[:])
                        if use_biases:
                            nc.scalar.activation(
                                out_slice,
                                out_slice,
                                IDENT,
                                bias=bv_sb[base : base + 64, a : a + 1],
                            )

        # ---- phase C: output projection + residual ---------------------
        with (
            tc.tile_pool(name="y_pool", bufs=4) as y_pool,
            tc.tile_pool(name="xres_pool", bufs=2) as xres_pool,
            tc.tile_pool(name="yt_psum", bufs=4, space="PSUM") as yt_psum,
        ):
            for mc in range(4):
                xc = xres_pool.tile([128, N], F32, name="xres", tag="xres")
                nc.sync.dma_start(out=xc[:], in_=x_d[bass.ts(mc, 128), :])
                for ic in range(2):
                    yp = yt_psum.tile([128, 512], F32, name="yt", tag="yt")
                    for kc in range(4):
                        nc.tensor.matmul(
                            yp[:],
                            wout_r[kc][:, bass.ts(mc, 128)],
                            resT[kc][:, bass.ts(ic, 512)],
                            start=(kc == 0),
                            stop=(kc == 3),
                        )
                    y_sb = y_pool.tile([128, 512], F32, name="ysb", tag="ysb")
                    if use_biases:
                        t1 = y_pool.tile([128, 512], F32, name="t1", tag="t1")
                        nc.scalar.activation(
                            t1[:], yp[:], IDENT, bias=bout_sb[:, mc : mc + 1]
                        )
                        nc.vector.tensor_add(y_sb[:], t1[:], xc[:, bass.ts(ic, 512)])
                    else:
                        nc.vector.tensor_add(y_sb[:], yp[:], xc[:, bass.ts(ic, 512)])
                    nc.sync.dma_start(
                        out=y_d[bass.ts(mc, 128), bass.ts(ic, 512)], in_=y_sb[:]
                    )


def _prep_shared(W_qkv, b_qkv, W_out, b_out):
    """Host-side weight/bias rearrangement into the kernel's layouts."""
    W_qkv = np.asarray(W_qkv, dtype=np.float32)
    b_qkv = np.asarray(b_qkv, dtype=np.float32)
    W_out = np.ascontiguousarray(np.asarray(W_out, dtype=np.float32))
    b_out = np.asarray(b_out, dtype=np.float32)

    q_cols = [W_qkv[:, 192 * h : 192 * h + 64] for h in range(NH)]
    k_cols = [W_qkv[:, 192 * h + 64 : 192 * h + 128] for h in range(NH)]
    v_cols = [W_qkv[:, 192 * h + 128 : 192 * h + 192] for h in range(NH)]
    wqk = np.ascontiguousarray(np.concatenate(q_cols + k_cols, axis=1))
    wv = np.ascontiguousarray(np.concatenate(v_cols, axis=1))

    bq = [b_qkv[192 * h : 192 * h + 64] for h in range(NH)]
    bk = [b_qkv[192 * h + 64 : 192 * h + 128] for h in range(NH)]
    bvs = [b_qkv[192 * h + 128 : 192 * h + 192] for h in range(NH)]
    bqk = np.ascontiguousarray(
        np.concatenate(bq + bk).reshape(8, 128).T.astype(np.float32)
    )
    bv = np.ascontiguousarray(np.concatenate(bvs).reshape(4, 128).T.astype(np.float32))
    bout = np.ascontiguousarray(b_out.reshape(4, 128).T.astype(np.float32))
    return wqk, wv, W_out, bqk, bv, bout


def kernel(x, W_qkv, b_qkv, W_out, b_out):
    x = np.asarray(x, dtype=np.float32)
    wqk, wv, wout, bqk, bv, bout = _prep_shared(W_qkv, b_qkv, W_out, b_out)
    use_biases = bool(np.any(bqk != 0) or np.any(bv != 0) or np.any(bout != 0))

    key = ("prog", use_biases)
    if key not in _cache:
        _cache[key] = build_program(use_biases)
    nc = _cache[key]

    xb = x.reshape(B, C, N)
    in_maps = [
        {
            "x": np.ascontiguousarray(xb[b]),
            "wqk": wqk,
            "wv": wv,
            "wout": wout,
            "bqk": bqk,
            "bv": bv,
            "bout": bout,
        }
        for b in range(B)
    ]
    res = run_bass_kernel_spmd(nc, in_maps, core_ids=list(range(N_CORES)))
    out = np.stack([res.results[b]["y"] for b in range(B)], axis=0)
    return out.reshape(B, C, H, W).astype(np.float32)


# revision 19
# speedup vs baseline: 1.0568x; 1.0568x over previous
"""AttentionBlock kernel for Trainium2, data-parallel over batch across 8 NeuronCores.

Reference computation (per batch element b, with x viewed as [C, N] where N = H*W):
    xf   = x^T                                 # [N, C] tokens
    qkv  = xf @ W_qkv + b_qkv                  # [N, 3*h*d], h=8 heads, d=64
    S_h  = (q_h @ k_h^T) * d^-0.5              # [N, N] per head
    A_h  = softmax(S_h, axis=keys)
    res  = concat_h(A_h @ v_h) @ W_out + b_out + xf
    out  = res^T                               # [C, N]

Everything on-device is computed in the transposed (feature-major) layout so no
transposes are ever needed: x arrives as [C, N], qk^T = W_qk^T @ x, scores are
built directly as S^T = k^T q (keys on partitions), the softmax denominator
falls out of an appended ones-column in the v matmul, and the output projection
produces y^T = [C, N] which is exactly the DRAM output layout.

Matmuls run in float32r (fp32 fast mode, ~3x PE throughput vs fp32 on TRN2).
Engine split: PE matmuls; ACT does only the exp; DVE does PSUM->SBUF epilogues,
reciprocal and normalization; GPSIMD does fp32->fp32r staging casts and the
softmax-denominator partition broadcast.
"""

import numpy as np

import concourse.bass as bass
import concourse.tile as tile
from concourse import bacc, mybir
from concourse.bass_interp import get_hw_module
from concourse.bass_utils import run_bass_kernel_spmd

F32 = mybir.dt.float32
F32R = mybir.dt.float32r
BF16 = mybir.dt.bfloat16
EXP = mybir.ActivationFunctionType.Exp
IDENT = mybir.ActivationFunctionType.Identity

N_CORES = 8
B, C, H, W = 8, 512, 32, 32
N = H * W          # 1024 tokens
NH, D = 8, 64      # heads, head dim
SCALE = D ** -0.5  # 0.125

_cache = {}


def build_program(use_biases: bool):
    nc = bacc.Bacc("TRN2", target_bir_lowering=False, debug=False, num_devices=N_CORES)

    x_d = nc.dram_tensor("x", [C, N], F32, kind="ExternalInput").ap()
    wqk_d = nc.dram_tensor("wqk", [C, 2 * NH * D], F32, kind="ExternalInput").ap()
    wv_d = nc.dram_tensor("wv", [C, NH * D], F32, kind="ExternalInput").ap()
    wout_d = nc.dram_tensor("wout", [NH * D, C], F32, kind="ExternalInput").ap()
    bqk_d = nc.dram_tensor("bqk", [128, 8], F32, kind="ExternalInput").ap()
    bv_d = nc.dram_tensor("bv", [128, 4], F32, kind="ExternalInput").ap()
    bout_d = nc.dram_tensor("bout", [128, 4], F32, kind="ExternalInput").ap()
    y_d = nc.dram_tensor("y", [C, N], F32, kind="ExternalOutput").ap()

    with tile.TileContext(nc) as tc:
        _emit(nc, tc, x_d, wqk_d, wv_d, wout_d, bqk_d, bv_d, bout_d, y_d, use_biases)
    nc.compile()
    nc.m = get_hw_module(nc.m)
    return nc


def _emit(nc, tc, x_d, wqk_d, wv_d, wout_d, bqk_d, bv_d, bout_d, y_d, use_biases):
    import contextlib

    ctx = contextlib.ExitStack()
    with ctx:
        persist = ctx.enter_context(tc.tile_pool(name="persist", bufs=1))

        # ---- persistent SBUF tiles -------------------------------------
        x_r = [persist.tile([128, N], F32R, name=f"xr{i}", tag=f"xr{i}") for i in range(4)]
        wqk_r = [persist.tile([128, 1024], F32R, name=f"wqk{i}", tag=f"wqk{i}") for i in range(4)]
        wv_r = [persist.tile([128, 512], F32R, name=f"wv{i}", tag=f"wv{i}") for i in range(4)]
        wout_r = [persist.tile([128, 512], F32R, name=f"wout{i}", tag=f"wout{i}") for i in range(4)]
        # v with an appended ones column per head: [n-chunk][128, head, 65]
        v_sb = [persist.tile([128, NH, D + 1], BF16, name=f"v{i}", tag=f"v{i}") for i in range(8)]
        resT = [persist.tile([128, N], F32R, name=f"resT{i}", tag=f"resT{i}") for i in range(4)]
        ones_f = persist.tile([128, 64], F32, tag="ones_f")
        nc.vector.memset(ones_f[:], 1.0)
        zero_r = persist.tile([128, 1024], F32R, tag="zero_r")
        nc.vector.memset(zero_r[:].bitcast(F32), 0.0)

        if use_biases:
            bqk_sb = persist.tile([128, 8], F32, tag="bqk")
            bv_sb = persist.tile([128, 4], F32, tag="bv")
            bout_sb = persist.tile([128, 4], F32, tag="bout")
            nc.sync.dma_start(out=bqk_sb[:], in_=bqk_d[:])
            nc.sync.dma_start(out=bv_sb[:], in_=bv_d[:])
            nc.sync.dma_start(out=bout_sb[:], in_=bout_d[:])

        for i in range(8):
            nc.vector.tensor_copy(
                v_sb[i][:, :, D : D + 1],
                ones_f[:, 0:NH].rearrange("p (h o) -> p h o", o=1),
            )

        # ---- fused pipeline: loads, projections, attention ------------------
        with (
            tc.tile_pool(name="stage", bufs=2) as stage,
            tc.tile_pool(name="qkT_pool", bufs=8) as qkT_pool,
            tc.tile_pool(name="vtmp_pool", bufs=2) as vtmp_pool,
            tc.tile_pool(name="exp_pool", bufs=16) as exp_pool,
            tc.tile_pool(name="z_pool", bufs=3) as z_pool,
            tc.tile_pool(name="big_psum", bufs=2, space="PSUM") as big_psum,
            tc.tile_pool(name="v_psum", bufs=1, space="PSUM") as v_psum,
            tc.tile_pool(name="rest_psum", bufs=3, space="PSUM") as rest_psum,
        ):
            def load_round(dst, dram_ap, width, eng):
                t = stage.tile([128, 1024], F32, name="stage", tag="stage")
                eng.dma_start(out=t[:, 0:width], in_=dram_ap)
                nc.vector.tensor_copy(dst, t[:, 0:width])

            for i in range(4):
                load_round(x_r[i][:], x_d[bass.ts(i, 128), :], 1024, nc.sync)
            for i in range(4):
                load_round(wqk_r[i][:], wqk_d[bass.ts(i, 128), :], 1024, nc.scalar)
            for i in range(4):
                load_round(wv_r[i][:], wv_d[bass.ts(i, 128), :], 512, nc.gpsimd)
            for i in range(4):
                load_round(wout_r[i][:], wout_d[bass.ts(i, 128), :], 512, nc.gpsimd)

            def emit_qkT(m):
                # qkT[g, n] = sum_c wqk[c, g] x[c, n].  The two heads in this
                # m-chunk land in separate tiles whose rows 64-127 are zero so
                # attention matmuls run with a full K=128 contraction (K=64
                # matmuls never register as PE activity and the clock stays
                # throttled at 1.2 GHz).
                te = qkT_pool.tile([128, N], F32R, name="qkTe", tag="qkT")
                to = qkT_pool.tile([128, N], F32R, name="qkTo", tag="qkT")
                qp = big_psum.tile([128, N], F32, name="qk", tag="big")
                for half in range(2):
                    for kc in range(4):
                        nc.tensor.matmul(
                            qp[:, bass.ts(half, 512)],
                            wqk_r[kc][:, bass.ts(m, 128)],
                            x_r[kc][:, bass.ts(half, 512)],
                            start=(kc == 0),
                            stop=(kc == 3),
                        )
                if use_biases:
                    nc.scalar.activation(
                        qp[:], qp[:], IDENT, bias=bqk_sb[:, m : m + 1]
                    )
                nc.vector.tensor_copy(te[0:64, :], qp[0:64, :])
                nc.vector.tensor_copy(to[0:64, :], qp[64:128, :])
                nc.scalar.dma_start(out=te[64:128, :], in_=zero_r[64:128, :])
                nc.scalar.dma_start(out=to[64:128, :], in_=zero_r[64:128, :])
                return te, to

            def emit_v():
                # v_all[n, h*64+d] = sum_c x[c, n] wv[c, h*64+d]
                for nc_i in range(8):
                    vp = v_psum.tile([128, 512], F32, name="vp", tag="vp")
                    for kc in range(4):
                        nc.tensor.matmul(
                            vp[:],
                            x_r[kc][:, bass.ts(nc_i, 128)],
                            wv_r[kc][:],
                            start=(kc == 0),
                            stop=(kc == 3),
                        )
                    vt = vtmp_pool.tile([128, 512], BF16, name="vt", tag="vt")
                    nc.scalar.activation(vt[:], vp[:], IDENT)
                    nc.gpsimd.tensor_copy(
                        v_sb[nc_i][:, :, 0:D],
                        vt[:].rearrange("p (h d) -> p h d", h=NH),
                    )

            qkT_next = (emit_qkT(0), emit_qkT(4))
            for a in range(4):  # head pair (2a, 2a+1)
                (q_e, q_o), (k_e, k_o) = qkT_next
                qk_h = {0: (q_e, k_e), 64: (q_o, k_o)}
                exps = {0: [], 64: []}
                # S^T = k^T q with zero-padded K=128 operands (full PE array)
                for jc in range(8):
                    sts = {}
                    for base in (0, 64):
                        sts[base] = big_psum.tile([128, N], F32, name="st", tag="big")
                    for base in (0, 64):
                        qt, kt = qk_h[base]
                        for ih in range(2):
                            nc.tensor.matmul(
                                sts[base][:, bass.ts(ih, 512)],
                                kt[:, bass.ts(jc, 128)],
                                qt[:, bass.ts(ih, 512)],
                                start=True,
                                stop=True,
                            )
                    for base in (0, 64):
                        e = exp_pool.tile([128, N], BF16, name="exp", tag="exp")
                        nc.scalar.activation(e[:], sts[base][:], EXP, scale=SCALE)
                        exps[base].append(e)
                if a < 3:
                    qkT_next = (emit_qkT(a + 1), emit_qkT(4 + a + 1))
                if a == 0:
                    emit_v()
                # res^T = v_aug . expS^T per head; psum row 64 = softmax denom Z
                for base in (0, 64):
                    h = 2 * a + (base // 64)
                    for ih in range(2):
                        rp = rest_psum.tile([D + 1, 512], F32, name="rest", tag="rest")
                        for jc in range(8):
                            nc.tensor.matmul(
                                rp[:],
                                v_sb[jc][:, h, :],
                                exps[base][jc][:, bass.ts(ih, 512)],
                                start=(jc == 0),
                                stop=(jc == 7),
                            )
                        # 1/Z: spread Z across 128 partitions so the DVE
                        # reciprocal runs on all lanes, then broadcast back.
                        zsb = z_pool.tile([1, 512], F32, name="zsb", tag="zsb")
                        nc.vector.tensor_copy(zsb[:], rp[D : D + 1, :])
                        zt = z_pool.tile([128, 4], F32, name="zt", tag="zt")
                        nc.sync.dma_start(
                            out=zt[:],
                            in_=zsb[:].rearrange("o (p f) -> o p f", p=128),
                        )
                        ztr = z_pool.tile([128, 4], F32, name="ztr", tag="ztr")
                        nc.vector.reciprocal(ztr[:], zt[:])
                        nc.sync.dma_start(
                            out=zsb[:].rearrange("o (p f) -> o p f", p=128),
                            in_=ztr[:],
                        )
                        zr = z_pool.tile([64, 512], F32, name="zr", tag="zr")
                        nc.gpsimd.partition_broadcast(zr[:], zsb[:])
                        out_slice = resT[a][base : base + 64, bass.ts(ih, 512)]
                        nc.vector.tensor_mul(out_slice, rp[0:D, :], zr[:])
                        if use_biases:
                            nc.scalar.activation(
                                out_slice,
                                out_slice,
                                IDENT,
                                bias=bv_sb[base : base + 64, a : a + 1],
                            )

        # ---- phase C: output projection + residual ---------------------
        with (
            tc.tile_pool(name="y_pool", bufs=4) as y_pool,
            tc.tile_pool(name="xres_pool", bufs=2) as xres_pool,
            tc.tile_pool(name="yt_psum", bufs=4, space="PSUM") as yt_psum,
        ):
            for mc in range(4):
                xc = xres_pool.tile([128, N], F32, name="xres", tag="xres")
                nc.sync.dma_start(out=xc[:], in_=x_d[bass.ts(mc, 128), :])
                for ic in range(2):
                    yp = yt_psum.tile([128, 512], F32, name="yt", tag="yt")
                    for kc in range(4):
                        nc.tensor.matmul(
                            yp[:],
                            wout_r[kc][:, bass.ts(mc, 128)],
                            resT[kc][:, bass.ts(ic, 512)],
                            start=(kc == 0),
                            stop=(kc == 3),
                        )
                    y_sb = y_pool.tile([128, 512], F32, name="ysb", tag="ysb")
                    if use_biases:
                        t1 = y_pool.tile([128, 512], F32, name="t1", tag="t1")
                        nc.scalar.activation(
                            t1[:], yp[:], IDENT, bias=bout_sb[:, mc : mc + 1]
                        )
                        nc.vector.tensor_add(y_sb[:], t1[:], xc[:, bass.ts(ic, 512)])
                    else:
                        nc.vector.tensor_add(y_sb[:], yp[:], xc[:, bass.ts(ic, 512)])
                    nc.sync.dma_start(
                        out=y_d[bass.ts(mc, 128), bass.ts(ic, 512)], in_=y_sb[:]
                    )


def _prep_shared(W_qkv, b_qkv, W_out, b_out):
    """Host-side weight/bias rearrangement into the kernel's layouts."""
    W_qkv = np.asarray(W_qkv, dtype=np.float32)
    b_qkv = np.asarray(b_qkv, dtype=np.float32)
    W_out = np.ascontiguousarray(np.asarray(W_out, dtype=np.float32))
    b_out = np.asarray(b_out, dtype=np.float32)

    q_cols = [W_qkv[:, 192 * h : 192 * h + 64] for h in range(NH)]
    k_cols = [W_qkv[:, 192 * h + 64 : 192 * h + 128] for h in range(NH)]
    v_cols = [W_qkv[:, 192 * h + 128 : 192 * h + 192] for h in range(NH)]
    wqk = np.ascontiguousarray(np.concatenate(q_cols + k_cols, axis=1))
    wv = np.ascontiguousarray(np.concatenate(v_cols, axis=1))

    bq = [b_qkv[192 * h : 192 * h + 64] for h in range(NH)]
    bk = [b_qkv[192 * h + 64 : 192 * h + 128] for h in range(NH)]
    bvs = [b_qkv[192 * h + 128 : 192 * h + 192] for h in range(NH)]
    bqk = np.ascontiguousarray(
        np.concatenate(bq + bk).reshape(8, 128).T.astype(np.float32)
    )
    bv = np.ascontiguousarray(np.concatenate(bvs).reshape(4, 128).T.astype(np.float32))
    bout = np.ascontiguousarray(b_out.reshape(4, 128).T.astype(np.float32))
    return wqk, wv, W_out, bqk, bv, bout


def kernel(x, W_qkv, b_qkv, W_out, b_out):
    x = np.asarray(x, dtype=np.float32)
    wqk, wv, wout, bqk, bv, bout = _prep_shared(W_qkv, b_qkv, W_out, b_out)
    use_biases = bool(np.any(bqk != 0) or np.any(bv != 0) or np.any(bout != 0))

    key = ("prog", use_biases)
    if key not in _cache:
        _cache[key] = build_program(use_biases)
    nc = _cache[key]

    xb = x.reshape(B, C, N)
    in_maps = [
        {
            "x": np.ascontiguousarray(xb[b]),
            "wqk": wqk,
            "wv": wv,
            "wout": wout,
            "bqk": bqk,
            "bv": bv,
            "bout": bout,
        }
        for b in range(B)
    ]
    res = run_bass_kernel_spmd(nc, in_maps, core_ids=list(range(N_CORES)))
    out = np.stack([res.results[b]["y"] for b in range(B)], axis=0)
    return out.reshape(B, C, H, W).astype(np.float32)


# revision 21
# speedup vs baseline: 1.2567x; 1.1892x over previous
"""AttentionBlock kernel for Trainium2, data-parallel over batch across 8 NeuronCores.

Reference computation (per batch element b, with x viewed as [C, N] where N = H*W):
    xf   = x^T                                 # [N, C] tokens
    qkv  = xf @ W_qkv + b_qkv                  # [N, 3*h*d], h=8 heads, d=64
    S_h  = (q_h @ k_h^T) * d^-0.5              # [N, N] per head
    A_h  = softmax(S_h, axis=keys)
    res  = concat_h(A_h @ v_h) @ W_out + b_out + xf
    out  = res^T                               # [C, N]

Everything on-device is computed in the transposed (feature-major) layout so no
transposes are ever needed: x arrives as [C, N], qk^T = W_qk^T @ x, scores are
built directly as S^T = k^T q (keys on partitions), the softmax denominator
falls out of an appended ones-column in the v matmul, and the output projection
produces y^T = [C, N] which is exactly the DRAM output layout.

Matmuls run in float32r (fp32 fast mode, ~3x PE throughput vs fp32 on TRN2).
Engine split: PE matmuls; ACT does only the exp; DVE does PSUM->SBUF epilogues,
reciprocal and normalization; GPSIMD does fp32->fp32r staging casts and the
softmax-denominator partition broadcast.
"""

import numpy as np

import concourse.bass as bass
import concourse.tile as tile
from concourse import bacc, mybir
from concourse.bass_interp import get_hw_module
from concourse.bass_utils import run_bass_kernel_spmd

F32 = mybir.dt.float32
F32R = mybir.dt.float32r
BF16 = mybir.dt.bfloat16
EXP = mybir.ActivationFunctionType.Exp
IDENT = mybir.ActivationFunctionType.Identity

N_CORES = 8
B, C, H, W = 8, 512, 32, 32
N = H * W          # 1024 tokens
NH, D = 8, 64      # heads, head dim
SCALE = D ** -0.5  # 0.125

_cache = {}


def build_program(use_biases: bool):
    nc = bacc.Bacc("TRN2", target_bir_lowering=False, debug=False, num_devices=N_CORES)

    x_d = nc.dram_tensor("x", [C, N], F32, kind="ExternalInput").ap()
    wqk_d = nc.dram_tensor("wqk", [C, 2 * NH * D], F32, kind="ExternalInput").ap()
    wv_d = nc.dram_tensor("wv", [C, NH * D], F32, kind="ExternalInput").ap()
    wout_d = nc.dram_tensor("wout", [NH * D, C], F32, kind="ExternalInput").ap()
    bqk_d = nc.dram_tensor("bqk", [128, 8], F32, kind="ExternalInput").ap()
    bv_d = nc.dram_tensor("bv", [128, 4], F32, kind="ExternalInput").ap()
    bout_d = nc.dram_tensor("bout", [128, 4], F32, kind="ExternalInput").ap()
    y_d = nc.dram_tensor("y", [C, N], F32, kind="ExternalOutput").ap()

    with tile.TileContext(nc) as tc:
        _emit(nc, tc, x_d, wqk_d, wv_d, wout_d, bqk_d, bv_d, bout_d, y_d, use_biases)
    nc.compile()
    nc.m = get_hw_module(nc.m)
    return nc


def _emit(nc, tc, x_d, wqk_d, wv_d, wout_d, bqk_d, bv_d, bout_d, y_d, use_biases):
    import contextlib

    ctx = contextlib.ExitStack()
    with ctx:
        persist = ctx.enter_context(tc.tile_pool(name="persist", bufs=1))

        # ---- persistent SBUF tiles -------------------------------------
        x_r = [persist.tile([128, N], F32R, name=f"xr{i}", tag=f"xr{i}") for i in range(4)]
        wqk_r = [persist.tile([128, 1024], F32R, name=f"wqk{i}", tag=f"wqk{i}") for i in range(4)]
        wv_r = [persist.tile([128, 512], F32R, name=f"wv{i}", tag=f"wv{i}") for i in range(4)]
        wout_r = [persist.tile([128, 512], F32R, name=f"wout{i}", tag=f"wout{i}") for i in range(4)]
        # v with an appended ones column per head: [n-chunk][128, head, 65]
        v_sb = [persist.tile([128, NH, D + 1], BF16, name=f"v{i}", tag=f"v{i}") for i in range(8)]
        resT = [persist.tile([128, N], F32R, name=f"resT{i}", tag=f"resT{i}") for i in range(4)]
        ones_f = persist.tile([128, 64], F32, tag="ones_f")
        nc.vector.memset(ones_f[:], 1.0)
        zero_r = persist.tile([128, 1024], F32R, tag="zero_r")
        nc.vector.memset(zero_r[:].bitcast(F32), 0.0)

        if use_biases:
            bqk_sb = persist.tile([128, 8], F32, tag="bqk")
            bv_sb = persist.tile([128, 4], F32, tag="bv")
            bout_sb = persist.tile([128, 4], F32, tag="bout")
            nc.sync.dma_start(out=bqk_sb[:], in_=bqk_d[:])
            nc.sync.dma_start(out=bv_sb[:], in_=bv_d[:])
            nc.sync.dma_start(out=bout_sb[:], in_=bout_d[:])

        for i in range(8):
            nc.vector.tensor_copy(
                v_sb[i][:, :, D : D + 1],
                ones_f[:, 0:NH].rearrange("p (h o) -> p h o", o=1),
            )

        # ---- fused pipeline: loads, projections, attention ------------------
        with (
            tc.tile_pool(name="stage", bufs=2) as stage,
            tc.tile_pool(name="qkT_pool", bufs=8) as qkT_pool,
            tc.tile_pool(name="vtmp_pool", bufs=2) as vtmp_pool,
            tc.tile_pool(name="exp_pool", bufs=33) as exp_pool,
            tc.tile_pool(name="z_pool", bufs=3) as z_pool,
            tc.tile_pool(name="big_psum", bufs=2, space="PSUM") as big_psum,
            tc.tile_pool(name="v_psum", bufs=1, space="PSUM") as v_psum,
            tc.tile_pool(name="rest_psum", bufs=3, space="PSUM") as rest_psum,
        ):
            def load_round(dst, dram_ap, width, eng):
                t = stage.tile([128, 1024], F32, name="stage", tag="stage")
                eng.dma_start(out=t[:, 0:width], in_=dram_ap)
                nc.vector.tensor_copy(dst, t[:, 0:width])

            for i in range(4):
                load_round(x_r[i][:], x_d[bass.ts(i, 128), :], 1024, nc.sync)
            for i in range(4):
                load_round(wqk_r[i][:], wqk_d[bass.ts(i, 128), :], 1024, nc.gpsimd)
            for i in range(4):
                load_round(wv_r[i][:], wv_d[bass.ts(i, 128), :], 512, nc.gpsimd)
            for i in range(4):
                load_round(wout_r[i][:], wout_d[bass.ts(i, 128), :], 512, nc.gpsimd)

            def emit_qkT(m):
                # qkT[g, n] = sum_c wqk[c, g] x[c, n].  The two heads in this
                # m-chunk land in separate tiles whose rows 64-127 are zero so
                # attention matmuls run with a full K=128 contraction (K=64
                # matmuls never register as PE activity and the clock stays
                # throttled at 1.2 GHz).
                te = qkT_pool.tile([128, N], F32R, name="qkTe", tag="qkT")
                to = qkT_pool.tile([128, N], F32R, name="qkTo", tag="qkT")
                qp = big_psum.tile([128, N], F32, name="qk", tag="big")
                for half in range(2):
                    for kc in range(4):
                        nc.tensor.matmul(
                            qp[:, bass.ts(half, 512)],
                            wqk_r[kc][:, bass.ts(m, 128)],
                            x_r[kc][:, bass.ts(half, 512)],
                            start=(kc == 0),
                            stop=(kc == 3),
                        )
                if use_biases:
                    nc.scalar.activation(
                        qp[:], qp[:], IDENT, bias=bqk_sb[:, m : m + 1]
                    )
                nc.vector.tensor_copy(te[0:64, :], qp[0:64, :])
                nc.vector.tensor_copy(to[0:64, :], qp[64:128, :])
                nc.scalar.dma_start(out=te[64:128, :], in_=zero_r[64:128, :])
                nc.scalar.dma_start(out=to[64:128, :], in_=zero_r[64:128, :])
                return te, to

            def emit_v():
                # v_all[n, h*64+d] = sum_c x[c, n] wv[c, h*64+d]
                for nc_i in range(8):
                    vp = v_psum.tile([128, 512], F32, name="vp", tag="vp")
                    for kc in range(4):
                        nc.tensor.matmul(
                            vp[:],
                            x_r[kc][:, bass.ts(nc_i, 128)],
                            wv_r[kc][:],
                            start=(kc == 0),
                            stop=(kc == 3),
                        )
                    vt = vtmp_pool.tile([128, 512], BF16, name="vt", tag="vt")
                    nc.scalar.activation(vt[:], vp[:], IDENT)
                    nc.gpsimd.tensor_copy(
                        v_sb[nc_i][:, :, 0:D],
                        vt[:].rearrange("p (h d) -> p h d", h=NH),
                    )

            def emit_ST(qk_pair):
                (q_e, q_o), (k_e, k_o) = qk_pair
                qk_h = {0: (q_e, k_e), 64: (q_o, k_o)}
                exps = {0: [], 64: []}
                # S^T = k^T q with zero-padded K=128 operands (full PE array)
                for jc in range(8):
                    sts = {}
                    for base in (0, 64):
                        sts[base] = big_psum.tile([128, N], F32, name="st", tag="big")
                    for base in (0, 64):
                        qt, kt = qk_h[base]
                        for ih in range(2):
                            nc.tensor.matmul(
                                sts[base][:, bass.ts(ih, 512)],
                                kt[:, bass.ts(jc, 128)],
                                qt[:, bass.ts(ih, 512)],
                                start=True,
                                stop=True,
                            )
                    for base in (0, 64):
                        e = exp_pool.tile([128, N], BF16, name="exp", tag="exp")
                        nc.scalar.activation(e[:], sts[base][:], EXP, scale=SCALE)
                        exps[base].append(e)
                return exps

            def emit_resT(a, exps):
                # res^T = v_aug . expS^T per head; psum row 64 = softmax denom Z
                for base in (0, 64):
                    h = 2 * a + (base // 64)
                    for ih in range(2):
                        rp = rest_psum.tile([D + 1, 512], F32, name="rest", tag="rest")
                        for jc in range(8):
                            nc.tensor.matmul(
                                rp[:],
                                v_sb[jc][:, h, :],
                                exps[base][jc][:, bass.ts(ih, 512)],
                                start=(jc == 0),
                                stop=(jc == 7),
                            )
                        # 1/Z: spread Z across 128 partitions so the DVE
                        # reciprocal runs on all lanes, then broadcast back.
                        zsb = z_pool.tile([1, 512], F32, name="zsb", tag="zsb")
                        nc.vector.tensor_copy(zsb[:], rp[D : D + 1, :])
                        zt = z_pool.tile([128, 4], F32, name="zt", tag="zt")
                        nc.sync.dma_start(
                            out=zt[:],
                            in_=zsb[:].rearrange("o (p f) -> o p f", p=128),
                        )
                        ztr = z_pool.tile([128, 4], F32, name="ztr", tag="ztr")
                        nc.vector.reciprocal(ztr[:], zt[:])
                        nc.sync.dma_start(
                            out=zsb[:].rearrange("o (p f) -> o p f", p=128),
                            in_=ztr[:],
                        )
                        zr = z_pool.tile([64, 512], F32, name="zr", tag="zr")
                        nc.gpsimd.partition_broadcast(zr[:], zsb[:])
                        out_slice = resT[a][base : base + 64, bass.ts(ih, 512)]
                        nc.vector.tensor_mul(out_slice, rp[0:D, :], zr[:])
                        if use_biases:
                            nc.scalar.activation(
                                out_slice,
                                out_slice,
                                IDENT,
                                bias=bv_sb[base : base + 64, a : a + 1],
                            )

            # two-stage software pipeline: scores/exp of pair a+1 are emitted
            # before the res^T of pair a so the ACT exp stream never starves.
            exps_cur = emit_ST((emit_qkT(0), emit_qkT(4)))
            for a in range(4):
                if a < 3:
                    exps_next = emit_ST((emit_qkT(a + 1), emit_qkT(4 + a + 1)))
                if a == 0:
                    emit_v()
                emit_resT(a, exps_cur)
                if a < 3:
                    exps_cur = exps_next

        # ---- phase C: output projection + residual ---------------------
        with (
            tc.tile_pool(name="y_pool", bufs=4) as y_pool,
            tc.tile_pool(name="xres_pool", bufs=2) as xres_pool,
            tc.tile_pool(name="yt_psum", bufs=4, space="PSUM") as yt_psum,
        ):
            for mc in range(4):
                xc = xres_pool.tile([128, N], F32, name="xres", tag="xres")
                nc.sync.dma_start(out=xc[:], in_=x_d[bass.ts(mc, 128), :])
                for ic in range(2):
                    yp = yt_psum.tile([128, 512], F32, name="yt", tag="yt")
                    for kc in range(4):
                        nc.tensor.matmul(
                            yp[:],
                            wout_r[kc][:, bass.ts(mc, 128)],
                            resT[kc][:, bass.ts(ic, 512)],
                            start=(kc == 0),
                            stop=(kc == 3),
                        )
                    y_sb = y_pool.tile([128, 512], F32, name="ysb", tag="ysb")
                    if use_biases:
                        t1 = y_pool.tile([128, 512], F32, name="t1", tag="t1")
                        nc.scalar.activation(
                            t1[:], yp[:], IDENT, bias=bout_sb[:, mc : mc + 1]
                        )
                        nc.vector.tensor_add(y_sb[:], t1[:], xc[:, bass.ts(ic, 512)])
                    else:
                        nc.vector.tensor_add(y_sb[:], yp[:], xc[:, bass.ts(ic, 512)])
                    nc.sync.dma_start(
                        out=y_d[bass.ts(mc, 128), bass.ts(ic, 512)], in_=y_sb[:]
                    )


def _prep_shared(W_qkv, b_qkv, W_out, b_out):
    """Host-side weight/bias rearrangement into the kernel's layouts."""
    W_qkv = np.asarray(W_qkv, dtype=np.float32)
    b_qkv = np.asarray(b_qkv, dtype=np.float32)
    W_out = np.ascontiguousarray(np.asarray(W_out, dtype=np.float32))
    b_out = np.asarray(b_out, dtype=np.float32)

    q_cols = [W_qkv[:, 192 * h : 192 * h + 64] for h in range(NH)]
    k_cols = [W_qkv[:, 192 * h + 64 : 192 * h + 128] for h in range(NH)]
    v_cols = [W_qkv[:, 192 * h + 128 : 192 * h + 192] for h in range(NH)]
    wqk = np.ascontiguousarray(np.concatenate(q_cols + k_cols, axis=1))
    wv = np.ascontiguousarray(np.concatenate(v_cols, axis=1))

    bq = [b_qkv[192 * h : 192 * h + 64] for h in range(NH)]
    bk = [b_qkv[192 * h + 64 : 192 * h + 128] for h in range(NH)]
    bvs = [b_qkv[192 * h + 128 : 192 * h + 192] for h in range(NH)]
    bqk = np.ascontiguousarray(
        np.concatenate(bq + bk).reshape(8, 128).T.astype(np.float32)
    )
    bv = np.ascontiguousarray(np.concatenate(bvs).reshape(4, 128).T.astype(np.float32))
    bout = np.ascontiguousarray(b_out.reshape(4, 128).T.astype(np.float32))
    return wqk, wv, W_out, bqk, bv, bout


def kernel(x, W_qkv, b_qkv, W_out, b_out):
    x = np.asarray(x, dtype=np.float32)
    wqk, wv, wout, bqk, bv, bout = _prep_shared(W_qkv, b_qkv, W_out, b_out)
    use_biases = bool(np.any(bqk != 0) or np.any(bv != 0) or np.any(bout != 0))

    key = ("prog", use_biases)
    if key not in _cache:
        _cache[key] = build_program(use_biases)
    nc = _cache[key]

    xb = x.reshape(B, C, N)
    in_maps = [
        {
            "x": np.ascontiguousarray(xb[b]),
            "wqk": wqk,
            "wv": wv,
            "wout": wout,
            "bqk": bqk,
            "bv": bv,
            "bout": bout,
        }
        for b in range(B)
    ]
    res = run_bass_kernel_spmd(nc, in_maps, core_ids=list(range(N_CORES)))
    out = np.stack([res.results[b]["y"] for b in range(B)], axis=0)
    return out.reshape(B, C, H, W).astype(np.float32)
